# revision 37
# baseline (speedup 1.0000x reference)
"""2-layer GAT on 8 Trainium2 NeuronCores — bf16 edition.

Strategy: dst-shard nodes across cores (6250 each, padded to 6272). Per layer:
each core computes node features h = x @ [W | W@att_src | W@att_dst] for its
shard (bf16), AllGathers the packed per-node rows into a replicated DRAM
table, then processes its own dst nodes in CSR supertiles (J tiles of 128
nodes; slots along the free dim): dma_gather pulls bf16 rows for every
incoming edge, attention (add/prelu/exp) runs batched per supertile, and the
alpha-weighted payload sum + denominator use axis-X reduce_sum over the
contiguous slot axis. int16 gather indices limit one gather to 32768 table
rows, so each tile gathers the lo/hi table halves separately into adjacent
slot ranges of one scr buffer; each gather is split into <=1024-row
sub-gathers spread across the 4 SWDGE queues with single_packet aggregation.
"""

import numpy as np

N = 50000
E = 800000
R = 8
NPC = N // R  # 6250 owned nodes per core
TPC = 49  # tiles of 128 nodes
NPAD = TPC * 128  # 6272 rows per shard
HALF = 4 * NPAD  # 25088 table rows per half
IN_CH = 128
HIDDEN = 32
HEADS = 4
OUT_CH = 64
NEG_SLOPE = 0.2
EL1 = 256  # L1 table row, bf16 elems: [h(128) | as(4) | pad], 512B
EL2 = 128  # L2 table row, bf16 elems: [h2(64) | as2(1) | pad], 256B
SENT = 6250  # sentinel row (first pad row of core 0 / core 4) in each half
CAP = 34  # supertile packing: J*(D_lo+D_hi) <= CAP


# ---------------------------------------------------------------- host planner
def _build_plan(edge_index):
    src = np.concatenate([edge_index[0], np.arange(N, dtype=np.int64)]).astype(np.int64)
    dst = np.concatenate([edge_index[1], np.arange(N, dtype=np.int64)]).astype(np.int64)
    lo = src < (N // 2)  # src owned by cores 0-3 -> table half 0

    # degree per (dst, half)
    d_lo = np.bincount(dst[lo], minlength=N)
    d_hi = np.bincount(dst[~lo], minlength=N)

    # per-core permutation: sort desc by degree profile; pads (deg 0) at end
    perms = []  # perms[c] = array of orig node ids, len NPC, permuted order
    pos = np.empty(N, dtype=np.int64)  # node id -> permuted global row
    for c in range(R):
        ids = np.arange(c * NPC, (c + 1) * NPC)
        key = np.maximum(d_lo[ids], d_hi[ids]) * 1000 + d_lo[ids] + d_hi[ids]
        order = np.argsort(-key, kind="stable")
        p = ids[order]
        perms.append(p)
        pos[p] = c * NPAD + np.arange(NPC)

    # shared per-tile max degrees (padded rows have degree 0)
    dlo_t = np.zeros(TPC, dtype=np.int64)
    dhi_t = np.zeros(TPC, dtype=np.int64)
    for c in range(R):
        dl = d_lo[perms[c]]
        dh = d_hi[perms[c]]
        dl = np.concatenate([dl, np.zeros(NPAD - NPC, dtype=np.int64)])
        dh = np.concatenate([dh, np.zeros(NPAD - NPC, dtype=np.int64)])
        dlo_t = np.maximum(dlo_t, dl.reshape(TPC, 128).max(axis=1))
        dhi_t = np.maximum(dhi_t, dh.reshape(TPC, 128).max(axis=1))
    dlo_t = np.maximum(dlo_t, 1)
    dhi_t = np.maximum(dhi_t, 1)

    # supertiles: group J consecutive tiles, J in {4,2,1}
    supers = []  # (t0, J, Dl, Dh)
    t = 0
    while t < TPC:
        for J in (4, 2, 1):
            if t + J <= TPC:
                Dl = int(dlo_t[t : t + J].max())
                Dh = int(dhi_t[t : t + J].max())
                if J * (Dl + Dh) <= CAP or J == 1:
                    break
        supers.append((t, J, Dl, Dh))
        t += J

    slots = sum(128 * J * (Dl + Dh) for (_, J, Dl, Dh) in supers)
    real = E + N
    plan = {
        "supers": supers,
        "perms": perms,
        "pos": pos,
        "inflation": slots * R / real,
    }

    # per-core slot tables: for owned dst nodes, list of src-pos per half
    gidx_cores = []
    for c in range(R):
        own = (dst >= c * NPC) & (dst < (c + 1) * NPC)
        s_own = src[own]
        d_own = dst[own]
        half = (~(s_own < (N // 2))).astype(np.int64)  # 0 lo, 1 hi
        lpos = pos[d_own] - c * NPAD  # local permuted pos of dst, [0, NPC)
        key = lpos * 2 + half
        order = np.argsort(key, kind="stable")
        key_s = key[order]
        sp = pos[s_own][order]  # permuted global row of src
        first = np.searchsorted(key_s, key_s)  # index of first occurrence
        rank = np.arange(len(key_s)) - first  # rank within (dst, half) group

        cols = []
        for t0, J, Dl, Dh in supers:
            n0 = t0 * 128
            n1 = (t0 + J) * 128
            for h, D, base, sent in ((0, Dl, 0, SENT), (1, Dh, HALF, HALF + SENT)):
                tab = np.full((n1 - n0, D), sent - base, dtype=np.int64)
                sel = (key_s % 2 == h) & (key_s // 2 >= n0) & (key_s // 2 < n1)
                rr = rank[sel]
                keep = rr < D
                tab[key_s[sel][keep] // 2 - n0, rr[keep]] = sp[sel][keep] - base
                assert keep.all(), "rank exceeded tile max degree"
                # gather layout: idx position (c*128 + p) -> partition p,
                # free block c. block c = j*D + k so g is [p, (j, k, row)].
                S = n1 - n0
                flat = np.empty(S * D, dtype=np.int16)
                nodes = np.arange(S)
                j = nodes // 128
                p = nodes % 128
                for k in range(D):
                    flat[(j * D + k) * 128 + p] = tab[nodes, k]
                wrapped = flat.reshape(-1, 16)  # [NI/16, 16]
                w = np.empty((128, S * D // 16), dtype=np.int16)
                for q in range(8):
                    w[q * 16 : (q + 1) * 16, :] = wrapped.T
                cols.append(w)
        gidx_cores.append(np.concatenate(cols, axis=1))
    plan["gidx"] = gidx_cores
    plan["W"] = gidx_cores[0].shape[1]
    assert all(g.shape[1] == plan["W"] for g in gidx_cores)
    return plan


# ---------------------------------------------------------------- bass kernel
def _build_bass(plan, phases="ABC"):
    import concourse.bacc as bacc
    import concourse.mybir as mybir
    import concourse.tile as tile
    from concourse.masks import make_identity

    f32 = mybir.dt.float32
    bf = mybir.dt.bfloat16
    i16 = mybir.dt.int16
    OP = mybir.AluOpType
    AF = mybir.ActivationFunctionType
    AX = mybir.AxisListType

    supers = plan["supers"]
    W = plan["W"]

    nc = bacc.Bacc(
        "TRN2",
        target_bir_lowering=False,
        debug=False,
        num_devices=R,
        num_swdge_queues=4,
        dynamic_dma_scratch_size=32768,
    )
    xT_in = nc.dram_tensor("xT", [128, NPAD], bf, kind="ExternalInput")
    gidx_in = nc.dram_tensor("gidx", [128, W], i16, kind="ExternalInput")
    wcat1_in = nc.dram_tensor("wcat1", [128, 136], bf, kind="ExternalInput")
    wcat2_in = nc.dram_tensor("wcat2", [128, 66], bf, kind="ExternalInput")
    b1_in = nc.dram_tensor("b1c", [128, 1], f32, kind="ExternalInput")
    b1n_in = nc.dram_tensor("b1n", [128, 1], f32, kind="ExternalInput")
    b2_in = nc.dram_tensor("b2r", [1, 64], f32, kind="ExternalInput")
    padneg_in = nc.dram_tensor("padneg", [128, 4], bf, kind="ExternalInput")
    csb_in = nc.dram_tensor("csb", [128, 2], f32, kind="ExternalInput")
    out_d = nc.dram_tensor("out", [NPAD, 64], f32, kind="ExternalOutput")

    with tile.TileContext(nc) as tc:
        with (
            tc.tile_pool(name="const", bufs=1) as cp,
            tc.tile_pool(name="work", bufs=5) as wp,
            tc.tile_pool(name="bigw", bufs=3) as wb,
            tc.tile_pool(name="gath", bufs=6) as gp,
            tc.tile_pool(name="psum", bufs=2, space="PSUM") as pp,
            tc.tile_pool(name="psumA", bufs=3, space="PSUM") as ppa,
            tc.tile_pool(name="dram", bufs=1, space="DRAM") as dp,
        ):
            shard1 = dp.tile([NPAD, EL1], bf)
            table1 = dp.tile([R * NPAD, EL1], bf, addr_space="Shared")
            shard2 = dp.tile([NPAD, EL2], bf)
            table2 = dp.tile([R * NPAD, EL2], bf, addr_space="Shared")

            wcat1 = cp.tile([128, 136], bf)
            nc.sync.dma_start(out=wcat1[:], in_=wcat1_in[:])
            wcat2 = cp.tile([128, 66], bf)
            nc.sync.dma_start(out=wcat2[:], in_=wcat2_in[:])
            b1c = cp.tile([128, 1], f32)
            nc.sync.dma_start(out=b1c[:], in_=b1_in[:])
            b1n = cp.tile([128, 1], f32)
            nc.sync.dma_start(out=b1n[:], in_=b1n_in[:])
            b2p = cp.tile([1, 64], f32)
            nc.sync.dma_start(out=b2p[:1, :], in_=b2_in[:])
            b2b = cp.tile([128, 64], f32)
            nc.gpsimd.partition_broadcast(b2b[:], b2p[:1, :])
            ident = cp.tile([128, 128], f32)
            make_identity(nc, ident[:])
            idxall = cp.tile([128, W], i16)
            nc.sync.dma_start(out=idxall[:], in_=gidx_in[:])
            sh1 = cp.tile([128, TPC * 132], bf)
            sh2 = cp.tile([128, TPC * 65], bf)
            adb1 = cp.tile([128, 4 * TPC], bf)
            adb2 = cp.tile([128, TPC], bf)
            padneg = cp.tile([128, 4], bf)
            nc.sync.dma_start(out=padneg[:], in_=padneg_in[:])
            csb = cp.tile([128, 2], f32)
            nc.sync.dma_start(out=csb[:], in_=csb_in[:])

            # ---------------- phase A: h1ext = x @ [W1|Ws1|Wd1] per owned tile
            for t in range(TPC):
                xt = wp.tile([128, 128], bf, tag="xt")
                nc.sync.dma_start(out=xt[:], in_=xT_in[:, t * 128 : (t + 1) * 128])
                psA = ppa.tile([128, 136], f32, tag="psA")
                nc.tensor.matmul(
                    psA[:], lhsT=xt[:], rhs=wcat1[:], start=True, stop=True
                )
                nc.scalar.copy(out=sh1[:, t * 132 : t * 132 + 132], in_=psA[:, 0:132])
                nc.scalar.copy(out=adb1[:, 4 * t : 4 * t + 4], in_=psA[:, 132:136])
            # pad rows: alpha_src = -1e30 so padded slots vanish in the softmax
            # (padneg is 0 for real rows, -1e30 for partitions 106..127)
            nc.vector.tensor_tensor(
                out=sh1[:, 48 * 132 + 128 : 48 * 132 + 132],
                in0=sh1[:, 48 * 132 + 128 : 48 * 132 + 132],
                in1=padneg[:],
                op=OP.add,
            )
            nc.sync.dma_start(
                out=shard1[:]
                .rearrange("(t p) r -> p t r", p=128)[:, :, 0:132],
                in_=sh1[:].rearrange("p (t r) -> p t r", r=132),
            )

            nc.gpsimd.collective_compute(
                "AllGather",
                mybir.AluOpType.bypass,
                replica_groups=[list(range(R))],
                ins=[shard1.opt()],
                outs=[table1.opt()],
            )

            if phases == "A":
                dbg = wp.tile([128, 64], f32, tag="dbg")
                nc.sync.dma_start(out=dbg[:], in_=table1[0:128, 0:64])
                nc.sync.dma_start(out=out_d[0:128, :], in_=dbg[:])

            # ---------------- phase B: layer-1 attention + aggregation
            col = [0]  # running idx column offset
            qrr = [0]  # SWDGE queue round-robin

            def gather_pair(table, EL, J, Dl, Dh):
                outs_ = []
                for D, base0, base1 in ((Dl, 0, HALF), (Dh, HALF, R * NPAD)):
                    C = J * D  # 128-row blocks
                    g = gp.tile([128, C * EL], bf, tag="g")
                    splits = -(-C // 8)  # sub-gathers <= 1024 rows
                    c0 = 0
                    for s in range(splits):
                        c1 = C * (s + 1) // splits
                        NI = 128 * (c1 - c0)
                        nc.gpsimd.dma_gather(
                            g[:, c0 * EL : c1 * EL].rearrange(
                                "p (c r) -> p c r", r=EL
                            ),
                            table[base0:base1, :],
                            idxall[:, col[0] : col[0] + NI // 16],
                            NI,
                            NI,
                            EL,
                            single_packet=True,
                            queue_num=qrr[0] % 4,
                        )
                        qrr[0] += 1
                        col[0] += NI // 16
                        c0 = c1
                    outs_.append(g)
                return outs_

            for t0, J, Dl, Dh in supers if "B" in phases else []:
                Kt = Dl + Dh
                glo, ghi = gather_pair(table1, EL1, J, Dl, Dh)
                if "G" in phases:  # gathers only: consume via a dummy copy
                    dbg2 = wp.tile([128, 64], f32, tag="dbg2")
                    nc.vector.tensor_copy(out=dbg2[:], in_=glo[:, 0:64])
                    nc.vector.tensor_copy(out=dbg2[:], in_=ghi[:, 0:64])
                    nc.sync.dma_start(
                        out=out_d[t0 * 128 : (t0 + 1) * 128, :], in_=dbg2[:]
                    )
                    continue
                # e = as[src] + ad[dst], batched over all J tiles and halves
                ebuf = wp.tile([128, J * 4 * Kt], bf, tag="ebuf")
                adv = (
                    adb1[:, 4 * t0 : 4 * (t0 + J)]
                    .rearrange("p (j h) -> p j h", h=4)
                    .unsqueeze(3)
                )
                for g, D, k0 in ((glo, Dl, 0), (ghi, Dh, Dl)):
                    nc.vector.tensor_tensor(
                        out=ebuf[:]
                        .rearrange("p (j h k) -> p j h k", j=J, h=4)[
                            :, :, :, k0 : k0 + D
                        ],
                        in0=g[:]
                        .rearrange("p (j k r) -> p j k r", j=J, r=EL1)[
                            :, :, :, 128:132
                        ]
                        .rearrange("p j k h -> p j h k"),
                        in1=adv.to_broadcast([128, J, 4, D]),
                        op=OP.add,
                    )
                lbuf = wp.tile([128, J * 4 * Kt], bf, tag="lbuf")
                nc.scalar.activation(lbuf[:], ebuf[:], AF.Prelu, alpha=NEG_SLOPE)
                exb = wp.tile([128, J * 4 * Kt], bf, tag="exb")
                nc.scalar.activation(exb[:], lbuf[:], AF.Exp)
                den = wp.tile([128, J * 4], f32, tag="den")
                nc.vector.reduce_sum(
                    out=den[:],
                    in_=exb[:].rearrange("p (jh k) -> p jh k", k=Kt),
                    axis=AX.X,
                )
                nc.vector.tensor_scalar_add(den[:], den[:], 1e-16)
                rden = wp.tile([128, J * 4], f32, tag="rden")
                nc.vector.reciprocal(rden[:], den[:])

                for j in range(J):
                    t = t0 + j
                    scr = wb.tile([128, 128 * Kt], bf, tag="scr")
                    exv = exb[:].rearrange("p (j h k) -> p j h k", j=J, h=4)
                    for g, D, k0 in ((glo, Dl, 0), (ghi, Dh, Dl)):
                        nc.vector.tensor_tensor(
                            out=scr[:]
                            .rearrange("p (h c k) -> p h c k", h=4, c=HIDDEN)[
                                :, :, :, k0 : k0 + D
                            ],
                            in0=g[:]
                            .rearrange("p (j k r) -> p j k r", j=J, r=EL1)[
                                :, j, :, 0:128
                            ]
                            .rearrange("p k (h c) -> p h c k", h=4),
                            in1=exv[:, j, :, k0 : k0 + D]
                            .unsqueeze(2)
                            .to_broadcast([128, 4, HIDDEN, D]),
                            op=OP.mult,
                        )
                    raw = wp.tile([128, 128], f32, tag="raw")
                    nc.vector.reduce_sum(
                        out=raw[:],
                        in_=scr[:].rearrange("p (hc k) -> p hc k", k=Kt),
                        axis=AX.X,
                    )
                    out1 = wp.tile([128, 128], f32, tag="out1")
                    nc.vector.tensor_tensor(
                        out=out1[:].rearrange("p (h c) -> p h c", h=4),
                        in0=raw[:].rearrange("p (h c) -> p h c", h=4),
                        in1=rden[:, 4 * j : 4 * j + 4]
                        .unsqueeze(2)
                        .to_broadcast([128, 4, HIDDEN]),
                        op=OP.mult,
                    )
                    if "R" in phases:  # stop after aggregation
                        nc.sync.dma_start(
                            out=out_d[t * 128 : (t + 1) * 128, :], in_=out1[:, 0:64]
                        )
                        continue
                    # transpose -> [c, n], ELU(z + b1), then @ [W2|Ws2|Wd2]
                    psT = pp.tile([128, 128], f32, tag="psT")
                    nc.tensor.transpose(psT[:], out1[:], ident[:])
                    # ELU: relu(z) + exp(-relu(-z)) - 1, z = psT + b1
                    rt = wp.tile([128, 128], f32, tag="rt")
                    nc.scalar.activation(rt[:], psT[:], AF.Relu, bias=b1c[:, :1])
                    mt = wp.tile([128, 128], f32, tag="mt")
                    nc.scalar.activation(
                        mt[:], psT[:], AF.Relu, scale=-1.0, bias=b1n[:, :1]
                    )
                    emt = wp.tile([128, 128], f32, tag="emt")
                    nc.scalar.activation(emt[:], mt[:], AF.Exp, scale=-1.0)
                    elub = wp.tile([128, 128], bf, tag="elub")
                    nc.vector.tensor_tensor(elub[:], rt[:], emt[:], op=OP.add)
                    ps2 = pp.tile([128, 66], f32, tag="ps2")
                    nc.tensor.matmul(
                        ps2[:], lhsT=elub[:], rhs=wcat2[:], start=True, stop=True
                    )
                    nc.scalar.copy(out=sh2[:, t * 65 : t * 65 + 64], in_=ps2[:, 0:64])
                    nc.scalar.activation(
                        sh2[:, t * 65 + 64 : t * 65 + 65],
                        ps2[:, 64:65],
                        AF.Identity,
                        bias=csb[:, 0:1],
                    )
                    nc.scalar.activation(
                        adb2[:, t : t + 1],
                        ps2[:, 65:66],
                        AF.Identity,
                        bias=csb[:, 1:2],
                    )
                    if "C" not in phases:
                        nc.sync.dma_start(
                            out=out_d[t * 128 : (t + 1) * 128, :], in_=out1[:, 0:64]
                        )

            if "C" in phases:
                nc.vector.tensor_tensor(
                    out=sh2[:, 48 * 65 + 64 : 48 * 65 + 65],
                    in0=sh2[:, 48 * 65 + 64 : 48 * 65 + 65],
                    in1=padneg[:, 0:1],
                    op=OP.add,
                )
                nc.sync.dma_start(
                    out=shard2[:]
                    .rearrange("(t p) r -> p t r", p=128)[:, :, 0:65],
                    in_=sh2[:].rearrange("p (t r) -> p t r", r=65),
                )
                nc.gpsimd.collective_compute(
                    "AllGather",
                    mybir.AluOpType.bypass,
                    replica_groups=[list(range(R))],
                    ins=[shard2.opt()],
                    outs=[table2.opt()],
                )

            # ---------------- phase C: layer-2 attention + aggregation
            col2 = col[0]
            col[0] = 0
            for t0, J, Dl, Dh in supers if "C" in phases else []:
                Kt = Dl + Dh
                glo, ghi = gather_pair(table2, EL2, J, Dl, Dh)
                e2 = wp.tile([128, J * Kt], bf, tag="e2")
                ad2v = adb2[:, t0 : t0 + J].unsqueeze(2).unsqueeze(3)
                for g, D, k0 in ((glo, Dl, 0), (ghi, Dh, Dl)):
                    nc.vector.tensor_tensor(
                        out=e2[:]
                        .rearrange("p (j k) -> p j k", j=J)[:, :, k0 : k0 + D]
                        .unsqueeze(3),
                        in0=g[:].rearrange("p (j k r) -> p j k r", j=J, r=EL2)[
                            :, :, :, 64:65
                        ],
                        in1=ad2v.to_broadcast([128, J, D, 1]),
                        op=OP.add,
                    )
                l2b = wp.tile([128, J * Kt], bf, tag="l2b")
                nc.scalar.activation(l2b[:], e2[:], AF.Prelu, alpha=NEG_SLOPE)
                ex2 = wp.tile([128, J * Kt], bf, tag="ex2")
                nc.scalar.activation(ex2[:], l2b[:], AF.Exp)
                den2 = wp.tile([128, J], f32, tag="den2")
                nc.vector.reduce_sum(
                    out=den2[:],
                    in_=ex2[:].rearrange("p (j k) -> p j k", k=Kt),
                    axis=AX.X,
                )
                nc.vector.tensor_scalar_add(den2[:], den2[:], 1e-16)
                rden2 = wp.tile([128, J], f32, tag="rden2")
                nc.vector.reciprocal(rden2[:], den2[:])

                outst = wp.tile([128, J * 64], f32, tag="outst")
                for j in range(J):
                    t = t0 + j
                    scr2 = wb.tile([128, 64 * Kt], bf, tag="scr2")
                    ex2v = ex2[:].rearrange("p (j k) -> p j k", j=J)
                    for g, D, k0 in ((glo, Dl, 0), (ghi, Dh, Dl)):
                        nc.vector.tensor_tensor(
                            out=scr2[:]
                            .rearrange("p (c k) -> p c k", c=64)[:, :, k0 : k0 + D],
                            in0=g[:]
                            .rearrange("p (j k r) -> p j k r", j=J, r=EL2)[
                                :, j, :, 0:64
                            ]
                            .rearrange("p k c -> p c k"),
                            in1=ex2v[:, j, k0 : k0 + D]
                            .unsqueeze(1)
                            .to_broadcast([128, 64, D]),
                            op=OP.mult,
                        )
                    raw2 = wp.tile([128, 64], f32, tag="raw2")
                    nc.vector.reduce_sum(
                        out=raw2[:],
                        in_=scr2[:].rearrange("p (c k) -> p c k", k=Kt),
                        axis=AX.X,
                    )
                    nc.scalar.activation(
                        outst[:, j * 64 : j * 64 + 64],
                        raw2[:],
                        AF.Identity,
                        scale=rden2[:, j : j + 1],
                    )
                    nc.vector.tensor_tensor(
                        out=outst[:, j * 64 : j * 64 + 64],
                        in0=outst[:, j * 64 : j * 64 + 64],
                        in1=b2b[:],
                        op=OP.add,
                    )
                nc.sync.dma_start(
                    out=out_d[:]
                    .rearrange("(t p) r -> p t r", p=128)[:, t0 : t0 + J, :],
                    in_=outst[:].rearrange("p (t r) -> p t r", r=64),
                )
            assert "C" not in phases or col[0] == col2

    nc.finalize()
    return nc


# ---------------------------------------------------------------- entry point
_cache = {}


def kernel(x, edge_index, W1, att_src1, att_dst1, b1, W2, att_src2, att_dst2, b2):
    import ml_dtypes

    from concourse.bass_utils import run_bass_kernel_spmd

    BF = ml_dtypes.bfloat16
    x = np.asarray(x, dtype=np.float32)
    edge_index = np.asarray(edge_index, dtype=np.int64)
    W1 = np.asarray(W1, dtype=np.float32)
    W2 = np.asarray(W2, dtype=np.float32)
    att_src1 = np.asarray(att_src1, dtype=np.float32)
    att_dst1 = np.asarray(att_dst1, dtype=np.float32)
    att_src2 = np.asarray(att_src2, dtype=np.float32)
    att_dst2 = np.asarray(att_dst2, dtype=np.float32)
    b1 = np.asarray(b1, dtype=np.float32)
    b2 = np.asarray(b2, dtype=np.float32)

    import os

    phases = os.environ.get("KERNEL_PHASES", "ABC")
    key = (hash(edge_index.tobytes()), phases)
    if "plan" not in _cache or _cache.get("key") != key:
        _cache["plan"] = _build_plan(edge_index)
        _cache["nc"] = _build_bass(_cache["plan"], phases)
        _cache["key"] = key
    plan = _cache["plan"]
    nc = _cache["nc"]

    # weight packing: as = x @ (W1 . att_src) etc.
    W1r = W1.reshape(IN_CH, HEADS, HIDDEN)
    Ws1 = np.einsum("khc,hc->kh", W1r, att_src1)  # [128, 4]
    Wd1 = np.einsum("khc,hc->kh", W1r, att_dst1)
    wcat1 = np.concatenate([W1, Ws1, Wd1], axis=1).astype(BF)  # [128, 136]
    Ws2 = W2 @ att_src2[0]  # [128]
    Wd2 = W2 @ att_dst2[0]
    wcat2 = np.concatenate([W2, Ws2[:, None], Wd2[:, None]], axis=1).astype(BF)
    cs = wcat2.astype(np.float32).sum(axis=0)  # ELU+1 correction: colsums
    b2eff = (b2 - cs[0:64]).astype(np.float32)

    csb_host = np.zeros((128, 2), dtype=np.float32)
    csb_host[:, 0] = -cs[64]
    csb_host[:, 1] = -cs[65]
    padneg_host = np.zeros((128, 4), dtype=BF)
    padneg_host[NPC % 128 :] = BF(-1e30)
    in_maps = []
    for c in range(R):
        xp = np.zeros((NPAD, IN_CH), dtype=np.float32)
        xp[:NPC] = x[plan["perms"][c]]
        in_maps.append(
            {
                "xT": np.ascontiguousarray(xp.T).astype(BF),
                "gidx": plan["gidx"][c],
                "wcat1": wcat1,
                "wcat2": wcat2,
                "b1c": b1.reshape(128, 1).astype(np.float32),
                "b1n": (-b1).reshape(128, 1).astype(np.float32),
                "b2r": b2eff.reshape(1, 64),
                "padneg": padneg_host,
                "csb": csb_host,
            }
        )

    res = run_bass_kernel_spmd(nc, in_maps, core_ids=list(range(R)))
    _cache["last_res"] = res
    out = np.empty((N, OUT_CH), dtype=np.float32)
    for c in range(R):
        out[plan["perms"][c]] = res.results[c]["out"][:NPC]
    return out
